# revision 4
# baseline (speedup 1.0000x reference)
"""DiffUnpool batched GEMM on 8 Trainium2 NeuronCores, bf16 fast path.

out[b] = S[b] @ x[b] for b in 0..15 (B=16, M=2048, K=256, N=256); A is
passed through unused and never touches the device.

Sharding: pure data parallel over the batch dim - 2 batches per core, no
communication.  Host side pre-transposes S to S^T[b, k, p, m] and casts both
operands to bf16 (PE streams bf16 at 1 cycle/row vs fp32's 4, and DMA bytes
halve; measured end-to-end rel err ~3e-3, well under the 2e-2 gate).

Per-core device kernel (DMA-bound at ~4.25 MB/rep aggregate traffic):
  - x is the STATIONARY operand: matmul(psum[c,m], lhsT=x[k, c-half],
    rhs=S^T[k, m-quarter]) computes (S@x)^T in [c, m] layout.  This cuts
    matmul count to 32/rep (512 moving rows each) and lets every DMA be a
    plain 2D transfer with >=512B contiguous descriptors:
      * 4 S^T loads of [128, 2048] (4 KB/descriptor),
      * 4 x loads of [128, 256] (512 B/descriptor),
      * 4 output stores of [128, 2048] bf16 (4 KB/descriptor).
    The [b, ch, c, m] output layout is untangled on the host for free.
  - PSUM: 8 banks of [128, 512] f32; each bank accumulates 2 matmuls (k=0
    start, k=1 stop), then is drained f32->bf16 into the store buffer by DVE
    and Pool in parallel (2 banks each per (b, ch) group) so no single
    engine's copy throughput caps the pipeline.
  - loads on the SP HWDGE queue, stores on the ACT queue (keeps the load
    ring free of head-of-line blocking behind stores waiting on copies).
  - PE warmup: dummy matmuls burn the HAM clock-gate ramp (cold 1.2 GHz ->
    warm 2.4 GHz) while the first input DMAs are in flight.
"""

import numpy as np

B, N_ORIG, N_POOL, C = 16, 2048, 256, 256
N_CORES = 8
B_PER_CORE = B // N_CORES

_cache: dict = {}


def _apply_multiwait_split_patch():
    """This walrus build rejects instructions with >1 sync wait (CoreV3
    setupSyncWait: "Too many sync wait commands"), but Tile's add_semaphores
    stage attaches several.  Post-process the serialized BIR: for each
    instruction with N>1 waits insert N-1 single-wait NoOps right before it
    on the same engine - per-engine program order preserves the semantics."""
    import orjson
    import concourse.bass as bass

    if getattr(bass.Bass, "_mwsplit_patched", False):
        return

    counter = [0]

    def split_multiwait(bir: dict) -> dict:
        for fn in bir.get("functions", []):
            for blk in fn.get("blocks", []):
                out = []
                changed = False
                for inst in blk.get("instructions", []):
                    si = inst.get("sync_info") or {}
                    waits = si.get("on_wait") or []
                    if len(waits) > 1:
                        changed = True
                        for w in waits[:-1]:
                            counter[0] += 1
                            out.append(
                                {
                                    "engine": inst["engine"],
                                    "ins": [],
                                    "outs": [],
                                    "name": f"I-mwsplit-{counter[0]}",
                                    "opcode": "NoOp",
                                    "debug": inst.get("debug", 0),
                                    "sync_info": {"on_update": [], "on_wait": [w]},
                                }
                            )
                        si["on_wait"] = [waits[-1]]
                    out.append(inst)
                if changed:
                    blk["instructions"] = out
        return bir

    orig_bytes = bass.Bass.to_json_bytes

    def to_json_bytes(self) -> bytes:
        return orjson.dumps(split_multiwait(orjson.loads(orig_bytes(self))))

    def to_json_str(self) -> str:
        return to_json_bytes(self).decode()

    def to_json(self) -> dict:
        return orjson.loads(to_json_bytes(self))

    bass.Bass.to_json_bytes = to_json_bytes
    bass.Bass.to_json_str = to_json_str
    bass.Bass.to_json = to_json
    bass.Bass._mwsplit_patched = True


def _build_nc(reps: int = 1):
    import concourse.bass as bass
    import concourse.mybir as mybir
    import concourse.tile as tile

    _apply_multiwait_split_patch()

    f32 = mybir.dt.float32
    bf16 = mybir.dt.bfloat16
    nc = bass.Bass()
    # Per-core: st = S^T[b, k, p, m] (contraction dim on partitions),
    # xs = x slices, od = (S@x)^T as [b, ch, c, m] (host untangles).
    st = nc.declare_dram_parameter(
        "st", [B_PER_CORE, 2, 128, N_ORIG], bf16, isOutput=False
    )
    xs = nc.declare_dram_parameter(
        "xs", [B_PER_CORE, N_POOL, C], bf16, isOutput=False
    )
    od = nc.declare_dram_parameter(
        "od", [B_PER_CORE, 2, 128, N_ORIG], bf16, isOutput=True
    )

    KT = N_POOL // 128  # contraction tiles per batch (2)
    CH = C // 128       # output-channel halves (2) - the stationary M dim
    MQ = N_ORIG // 512  # moving m quarters (4) - one PSUM bank each

    with tile.TileContext(nc) as tc:
        with (
            tc.tile_pool(name="w", bufs=2 * B_PER_CORE * KT) as wpool,
            tc.tile_pool(name="xp", bufs=2 * B_PER_CORE * KT) as xpool,
            tc.tile_pool(name="ps", bufs=7, space="PSUM") as pspool,
            tc.tile_pool(name="wps", bufs=1, space="PSUM") as wpspool,
            tc.tile_pool(name="ob", bufs=2 * B_PER_CORE * CH) as opool,
            tc.tile_pool(name="wu", bufs=1) as wupool,
        ):
            # PE warmup: dummy matmuls into a scratch PSUM bank while the
            # first input DMAs are in flight, so the HAM clock-gate ramp
            # (cold 1.2 GHz -> warm 2.4 GHz) burns off before real matmuls.
            dummy_w = wupool.tile([128, 128], f32, tag="wu_w")
            dummy_x = wupool.tile([128, 64], f32, tag="wu_x")
            nc.gpsimd.memset(dummy_w[:], 1.0)
            nc.gpsimd.memset(dummy_x[:], 1.0)
            wps = wpspool.tile([128, 64], f32)
            NWU = 16
            for i in range(NWU):
                nc.tensor.matmul(
                    wps[:], dummy_w[:], dummy_x[:], start=(i == 0), stop=(i == NWU - 1)
                )
            for _ in range(reps):
                xt = {}
                wt = {}
                for b in range(B_PER_CORE):
                    for k in range(KT):
                        xk = xpool.tile([128, C], bf16, tag="x")
                        nc.sync.dma_start(
                            out=xk[:], in_=xs[b, k * 128 : (k + 1) * 128, :]
                        )
                        xt[(b, k)] = xk
                    for k in range(KT):
                        w = wpool.tile([128, N_ORIG], bf16, tag="w")
                        nc.sync.dma_start(out=w[:], in_=st[b, k])
                        wt[(b, k)] = w
                for b in range(B_PER_CORE):
                    for ch in range(CH):
                        ps = [
                            pspool.tile([128, 512], f32, tag="ps", name="ps")
                            for _ in range(MQ)
                        ]
                        for k in range(KT):
                            lhs = xt[(b, k)][:, ch * 128 : (ch + 1) * 128]
                            for mq in range(MQ):
                                nc.tensor.matmul(
                                    ps[mq][:],
                                    lhs,
                                    wt[(b, k)][:, mq * 512 : (mq + 1) * 512],
                                    start=(k == 0),
                                    stop=(k == KT - 1),
                                )
                        ob = opool.tile([128, N_ORIG], bf16, tag="ob")
                        for mq in range(MQ):
                            eng = nc.vector if mq % 2 == 0 else nc.gpsimd
                            eng.tensor_copy(
                                ob[:, mq * 512 : (mq + 1) * 512], ps[mq][:]
                            )
                        # stores on the ACT HWDGE queue: keeps the SP queue
                        # free for loads (in-order issue would head-of-line
                        # block the next rep's loads behind stores).
                        nc.scalar.dma_start(out=od[b, ch], in_=ob[:])
    return nc


def _get_nc():
    if "nc" not in _cache:
        _cache["nc"] = _build_nc()
    return _cache["nc"]


def _host_inputs(x: np.ndarray, S: np.ndarray):
    import ml_dtypes

    bf16 = ml_dtypes.bfloat16
    # S^T[b, k, p, m] = S[b, m, 128k+p]
    st_full = np.ascontiguousarray(S.transpose(0, 2, 1)).reshape(
        B, 2, 128, N_ORIG
    ).astype(bf16)
    x_full = np.ascontiguousarray(x).astype(bf16)
    return st_full, x_full


def _run(x: np.ndarray, S: np.ndarray, trace: bool = False):
    from concourse.bass_utils import run_bass_kernel_spmd

    nc = _get_nc()
    st_full, x_full = _host_inputs(x, S)
    core_ids = list(range(N_CORES))
    in_maps = [
        {
            "st": st_full[i * B_PER_CORE : (i + 1) * B_PER_CORE],
            "xs": x_full[i * B_PER_CORE : (i + 1) * B_PER_CORE],
        }
        for i in core_ids
    ]
    res = run_bass_kernel_spmd(nc, in_maps, core_ids, trace=trace)
    # od[b, ch, c, m] -> out[b, m, 128ch+c]
    dev = np.concatenate([res.results[i]["od"] for i in core_ids], axis=0)
    out = (
        dev.transpose(0, 3, 1, 2)
        .reshape(B, N_ORIG, C)
        .astype(np.float32)
    )
    return out, res


def kernel(x: np.ndarray, S: np.ndarray, A: np.ndarray = None, **_: dict) -> np.ndarray:
    x = np.asarray(x, dtype=np.float32)
    S = np.asarray(S, dtype=np.float32)
    out, _res = _run(x, S, trace=False)
    return out


# revision 7
# speedup vs baseline: 1.2672x; 1.2672x over previous
"""DiffUnpool batched GEMM on 8 Trainium2 NeuronCores, bf16 fast path.

out[b] = S[b] @ x[b] for b in 0..15 (B=16, M=2048, K=256, N=256); A is
passed through unused and never touches the device.

Sharding: pure data parallel over the batch dim - 2 batches per core, no
communication.  Host side pre-transposes S to S^T[b, k, p, m] and casts both
operands to bf16 (PE streams bf16 at 1 cycle/row vs fp32's 4, and DMA bytes
halve; measured end-to-end rel err ~3e-3, well under the 2e-2 gate).

Per-core device kernel (DMA-bound at ~4.25 MB/rep aggregate traffic):
  - x is the STATIONARY operand: matmul(psum[c,m], lhsT=x[k, c-half],
    rhs=S^T[k, m-quarter]) computes (S@x)^T in [c, m] layout.  This cuts
    matmul count to 32/rep (512 moving rows each) and lets every DMA be a
    plain 2D transfer with >=512B contiguous descriptors:
      * 4 S^T loads of [128, 2048] (4 KB/descriptor),
      * 4 x loads of [128, 256] (512 B/descriptor),
      * 4 output stores of [128, 2048] bf16 (4 KB/descriptor).
    The [b, ch, c, m] output layout is untangled on the host for free.
  - PSUM: 8 banks of [128, 512] f32; each bank accumulates 2 matmuls (k=0
    start, k=1 stop), then is drained f32->bf16 into the store buffer by DVE
    and Pool in parallel (2 banks each per (b, ch) group) so no single
    engine's copy throughput caps the pipeline.
  - loads on the SP HWDGE queue, stores on the ACT queue (keeps the load
    ring free of head-of-line blocking behind stores waiting on copies).
  - PE warmup: dummy matmuls burn the HAM clock-gate ramp (cold 1.2 GHz ->
    warm 2.4 GHz) while the first input DMAs are in flight.
"""

import numpy as np

B, N_ORIG, N_POOL, C = 16, 2048, 256, 256
N_CORES = 8
B_PER_CORE = B // N_CORES

_cache: dict = {}


def _apply_multiwait_split_patch():
    """This walrus build rejects instructions with >1 sync wait (CoreV3
    setupSyncWait: "Too many sync wait commands"), but Tile's add_semaphores
    stage attaches several.  Post-process the serialized BIR: for each
    instruction with N>1 waits insert N-1 single-wait NoOps right before it
    on the same engine - per-engine program order preserves the semantics."""
    import orjson
    import concourse.bass as bass

    if getattr(bass.Bass, "_mwsplit_patched", False):
        return

    counter = [0]

    def split_multiwait(bir: dict) -> dict:
        for fn in bir.get("functions", []):
            for blk in fn.get("blocks", []):
                out = []
                changed = False
                for inst in blk.get("instructions", []):
                    si = inst.get("sync_info") or {}
                    waits = si.get("on_wait") or []
                    if len(waits) > 1:
                        changed = True
                        for w in waits[:-1]:
                            counter[0] += 1
                            out.append(
                                {
                                    "engine": inst["engine"],
                                    "ins": [],
                                    "outs": [],
                                    "name": f"I-mwsplit-{counter[0]}",
                                    "opcode": "NoOp",
                                    "debug": inst.get("debug", 0),
                                    "sync_info": {"on_update": [], "on_wait": [w]},
                                }
                            )
                        si["on_wait"] = [waits[-1]]
                    out.append(inst)
                if changed:
                    blk["instructions"] = out
        return bir

    orig_bytes = bass.Bass.to_json_bytes

    def to_json_bytes(self) -> bytes:
        return orjson.dumps(split_multiwait(orjson.loads(orig_bytes(self))))

    def to_json_str(self) -> str:
        return to_json_bytes(self).decode()

    def to_json(self) -> dict:
        return orjson.loads(to_json_bytes(self))

    bass.Bass.to_json_bytes = to_json_bytes
    bass.Bass.to_json_str = to_json_str
    bass.Bass.to_json = to_json
    bass.Bass._mwsplit_patched = True


def _build_nc(
    reps: int = 1,
    warmup: int = 16,
    s_chunk: int = 2048,
    o_chunk: int = 2048,
):
    import concourse.bass as bass
    import concourse.mybir as mybir
    import concourse.tile as tile

    _apply_multiwait_split_patch()

    f32 = mybir.dt.float32
    bf16 = mybir.dt.bfloat16
    nc = bass.Bass()
    # Per-core: st = S^T[b, k, p, m] (contraction dim on partitions),
    # xs = x slices, od = (S@x)^T as [b, ch, c, m] (host untangles).
    st = nc.declare_dram_parameter(
        "st", [B_PER_CORE, 2, 128, N_ORIG], bf16, isOutput=False
    )
    xs = nc.declare_dram_parameter(
        "xs", [B_PER_CORE, N_POOL, C], bf16, isOutput=False
    )
    od = nc.declare_dram_parameter(
        "od", [B_PER_CORE, 2, 128, N_ORIG], bf16, isOutput=True
    )

    KT = N_POOL // 128  # contraction tiles per batch (2)
    CH = C // 128       # output-channel halves (2) - the stationary M dim
    MQ = N_ORIG // 512  # moving m quarters (4) - one PSUM bank each

    with tile.TileContext(nc) as tc:
        with (
            tc.tile_pool(name="w", bufs=2 * B_PER_CORE * KT) as wpool,
            tc.tile_pool(name="xp", bufs=2 * B_PER_CORE * KT) as xpool,
            tc.tile_pool(name="ps", bufs=7, space="PSUM") as pspool,
            tc.tile_pool(name="wps", bufs=1, space="PSUM") as wpspool,
            tc.tile_pool(name="ob", bufs=2 * B_PER_CORE * CH) as opool,
            tc.tile_pool(name="wu", bufs=1) as wupool,
        ):
            # PE warmup: dummy matmuls into a scratch PSUM bank while the
            # first input DMAs are in flight, so the HAM clock-gate ramp
            # (cold 1.2 GHz -> warm 2.4 GHz) burns off before real matmuls.
            if warmup:
                dummy_w = wupool.tile([128, 128], f32, tag="wu_w")
                dummy_x = wupool.tile([128, 64], f32, tag="wu_x")
                nc.gpsimd.memset(dummy_w[:], 1.0)
                nc.gpsimd.memset(dummy_x[:], 1.0)
                wps = wpspool.tile([128, 64], f32)
                for i in range(warmup):
                    nc.tensor.matmul(
                        wps[:],
                        dummy_w[:],
                        dummy_x[:],
                        start=(i == 0),
                        stop=(i == warmup - 1),
                    )
            NSC = N_ORIG // s_chunk  # S chunks per (b, k) tile
            for _ in range(reps):
                xt = {}
                wt = {}
                for b in range(B_PER_CORE):
                    for k in range(KT):
                        xk = xpool.tile([128, C], bf16, tag="x")
                        nc.sync.dma_start(
                            out=xk[:], in_=xs[b, k * 128 : (k + 1) * 128, :]
                        )
                        xt[(b, k)] = xk
                    for k in range(KT):
                        for sc in range(NSC):
                            w = wpool.tile([128, s_chunk], bf16, tag="w", name="w")
                            nc.sync.dma_start(
                                out=w[:],
                                in_=st[b, k, :, sc * s_chunk : (sc + 1) * s_chunk],
                            )
                            wt[(b, k, sc)] = w
                for b in range(B_PER_CORE):
                    for ch in range(CH):
                        ps = [
                            pspool.tile([128, 512], f32, tag="ps", name="ps")
                            for _ in range(MQ)
                        ]
                        for k in range(KT):
                            lhs = xt[(b, k)][:, ch * 128 : (ch + 1) * 128]
                            for mq in range(MQ):
                                sc, off = divmod(mq * 512, s_chunk)
                                nc.tensor.matmul(
                                    ps[mq][:],
                                    lhs,
                                    wt[(b, k, sc)][:, off : off + 512],
                                    start=(k == 0),
                                    stop=(k == KT - 1),
                                )
                        ob = opool.tile([128, N_ORIG], bf16, tag="ob")
                        for mq in range(MQ):
                            dst = ob[:, mq * 512 : (mq + 1) * 512]
                            if mq % 2 == 0:
                                nc.vector.tensor_copy(dst, ps[mq][:])
                            else:
                                nc.scalar.copy(dst, ps[mq][:])
                        # stores on the ACT HWDGE queue: keeps the SP queue
                        # free for loads (in-order issue would head-of-line
                        # block the next rep's loads behind stores).
                        for oc in range(N_ORIG // o_chunk):
                            nc.scalar.dma_start(
                                out=od[b, ch, :, oc * o_chunk : (oc + 1) * o_chunk],
                                in_=ob[:, oc * o_chunk : (oc + 1) * o_chunk],
                            )
    return nc


def _get_nc():
    if "nc" not in _cache:
        _cache["nc"] = _build_nc()
    return _cache["nc"]


def _host_inputs(x: np.ndarray, S: np.ndarray):
    import ml_dtypes

    bf16 = ml_dtypes.bfloat16
    # S^T[b, k, p, m] = S[b, m, 128k+p]
    st_full = np.ascontiguousarray(S.transpose(0, 2, 1)).reshape(
        B, 2, 128, N_ORIG
    ).astype(bf16)
    x_full = np.ascontiguousarray(x).astype(bf16)
    return st_full, x_full


def _run(x: np.ndarray, S: np.ndarray, trace: bool = False):
    from concourse.bass_utils import run_bass_kernel_spmd

    nc = _get_nc()
    st_full, x_full = _host_inputs(x, S)
    core_ids = list(range(N_CORES))
    in_maps = [
        {
            "st": st_full[i * B_PER_CORE : (i + 1) * B_PER_CORE],
            "xs": x_full[i * B_PER_CORE : (i + 1) * B_PER_CORE],
        }
        for i in core_ids
    ]
    res = run_bass_kernel_spmd(nc, in_maps, core_ids, trace=trace)
    # od[b, ch, c, m] -> out[b, m, 128ch+c]
    dev = np.concatenate([res.results[i]["od"] for i in core_ids], axis=0)
    out = (
        dev.transpose(0, 3, 1, 2)
        .reshape(B, N_ORIG, C)
        .astype(np.float32)
    )
    return out, res


def kernel(x: np.ndarray, S: np.ndarray, A: np.ndarray = None, **_: dict) -> np.ndarray:
    x = np.asarray(x, dtype=np.float32)
    S = np.asarray(S, dtype=np.float32)
    out, _res = _run(x, S, trace=False)
    return out
